# revision 40
# baseline (speedup 1.0000x reference)
"""LIF spiking-neuron layer on 8 Trainium2 NeuronCores (Bass/Tile).

Reference semantics (per neuron, T=6 steps, v0=0):
    v = v*0.5 + x_t ; s = (v >= 1.0) ; v = v - s
Output: spikes [T, B, C, H, W] float32 (values are exactly 0.0 / 1.0).

Sharding: data-parallel over batch (axis 1): 64 batches / 8 cores.
Per core the neuron field (8*128*32*32 = 1,048,576 elements) is laid
out as [128 partitions, 8192 cols], processed in column blocks that are
software-pipelined with a one-timestep skew.

Scaled-state formulation (bit-identical to the fp32 reference):
  state U_t = 2^t * v_t; host pre-scales inputs xs_t = 2^t * x_t
  (exact power-of-2 scalings commute with fp32 round-to-nearest).
  Per step:
    w_t = (U_t >= 2^t) * 2^t    engine per (blk, t):
                                  'v' DVE tensor_scalar (2x perf mode)
                                  'a' ScalarE Sign+Relu pair: with bias
                                      -(2^t - 2^(t-24)) the largest fp32
                                      below 2^t maps to sign(0)=0 and the
                                      near-threshold subtract is Sterbenz-
                                      exact, so this is exact for ALL
                                      fp32 inputs
                                  'l' Pool tensor_scalar
    U  -=  w_t                  'd' DVE / 'l' Pool tensor_tensor /
                                'p'/'f' PE identity matmuls
    U  +=  xs_{t+1}             by the load DMA itself (SWDGE CCE add),
                                or a DVE/Pool add from a staged
                                dependency-free load (LIF_STAGE)
  The TensorEngine packs the 6 fp8 spike planes into a 6-bit code
  C = sum_t 2^t s_t via identity matmuls accumulated in PSUM; ScalarE
  copies PSUM->SBUF as uint8; the store writes 1 MiB/core.  The host
  unpacks bits to {0,1} f32.

Schedule (TimelineSim 86127 ns vs 88360 baseline; DMA busy 72.9 us):
  descending blocks [1536,...,384], starts [0..6,6] (last two blocks
  bunched), t0 subs of blocks 1,3,4,5,6 + t2 sub of block 2 on Pool,
  spike compares (2,t2),(2,t5),(3,t1),(3,t2),(3,t5) on ScalarE, x_1 of
  block 0 staged (breaks the fill-phase accum->compare latency loop),
  and per-step packing for the last 2 blocks (NLAST=2).  All remaining
  DMA idle is the ~1.9 us issue ramp at the start, ~2 us of fill, and
  the ~5 us drain cascade (last accum + 900ns sem + compare/pack/copy/
  store chain).
"""

import os
import sys

import numpy as np

sys.path.insert(0, "/opt/trn_rl_repo")

import concourse.bacc as bacc
import concourse.bass as bass
import concourse.mybir as mybir
from concourse import tile
from concourse.bass_utils import run_bass_kernel_spmd
from concourse.masks import make_identity

T = 6
B = 64
C = 128
H = 32
W = 32
N_CORES = 8
B_PER_CORE = B // N_CORES
N_PER_CORE = B_PER_CORE * C * H * W  # 1,048,576
P = 128
FTOT = N_PER_CORE // P               # 8192
if os.environ.get("LIF_BLOCKS"):
    BLOCKS = [int(v) for v in os.environ["LIF_BLOCKS"].split(",")]
    assert sum(BLOCKS) == FTOT, BLOCKS
elif os.environ.get("LIF_NBLK"):
    _n = int(os.environ["LIF_NBLK"])
    BLOCKS = [FTOT // _n] * _n
else:
    BLOCKS = [1536, 1280, 1024, 1152, 1024, 1024, 768, 384]
NBLK = len(BLOCKS)
OFFS = [sum(BLOCKS[:i]) for i in range(NBLK)]
MM = 512                             # PE moving-free / PSUM chunk
SKEW = int(os.environ.get("LIF_SKEW", "1"))
REV = int(os.environ.get("LIF_REV", "1"))

# Subtract-engine pattern: one char per t in 0..4 ('d'=DVE, 'p'=PE,
# 'l'=Pool, 'f'=PE-fused sub+add consuming a staged x_{t+1} plane),
# applied to every block; per-block override via
# LIF_SUBPAT_B="blk:pattern;..."
SUBPAT = os.environ.get("LIF_SUBPAT", "ddddd")
assert len(SUBPAT) == T - 1 and set(SUBPAT) <= set("dplf"), SUBPAT
SUBPAT_B = {1: "ldddd", 2: "ddldd", 3: "ldddd", 4: "ldddd", 5: "ldddd",
            6: "ldddd"}
if os.environ.get("LIF_SUBPAT_B") is not None:
    SUBPAT_B = {}
    for it in os.environ.get("LIF_SUBPAT_B", "").split(";"):
        if it:
            b_, pat_ = it.split(":")
            assert len(pat_) == T - 1 and set(pat_) <= set("dplf")
            SUBPAT_B[int(b_)] = pat_

# How many steps ahead of use a staged plane's load is authored.
STAGE_LEAD = int(os.environ.get("LIF_STAGE_LEAD", "2"))
# Scheduler time hint: staged load for (blk, tl) is pinned to fire no
# earlier than (starts[blk] + tl - STAGE_LEAD) * STAGE_ROUND_US.
STAGE_ROUND_US = float(os.environ.get("LIF_STAGE_ROUND_US", "0"))


def sub_engine(blk, t):
    return SUBPAT_B.get(blk, SUBPAT)[t]


# Staged timesteps: "blk:ts[:eng];..." e.g. "6:45:l;7:12345" stages
# x_4,x_5 of block 6 (adds on Pool) and x_1..x_5 of block 7 (adds on
# DVE, the default) as dependency-free loads; the U += xs for those
# steps runs as a tensor_tensor add instead of an accum DMA,
# collapsing the drain-chain latency.
STAGED = {0: {1}}
STAGED_ENG = {0: "d"}
if os.environ.get("LIF_STAGE") is not None:
    STAGED = {}
    STAGED_ENG = {}
    for _it in os.environ.get("LIF_STAGE", "").split(";"):
        if _it:
            _parts = _it.split(":")
            _b = int(_parts[0])
            STAGED[_b] = {int(ch) for ch in _parts[1]}
            STAGED_ENG[_b] = _parts[2] if len(_parts) > 2 else "d"

# 'f' sub steps consume staged x_{t+1}: add them to the staged set
for _b in range(NBLK):
    for _t in range(T - 1):
        if SUBPAT_B.get(_b, SUBPAT)[_t] == "f":
            STAGED.setdefault(_b, set()).add(_t + 1)
            STAGED_ENG.setdefault(_b, "d")

# Spike-compare engine per (blk, t in 0..5): 'v' DVE (2x perf mode),
# 'l' Pool, 'a' ScalarE/Activation via Sign+Relu.  The 'a' path is
# exact: with bias -(2^t - 2^(t-24)), the largest fp32 below 2^t maps
# to sign(0)=0 (no spike) and every U >= 2^t maps to sign(+)=1; the
# near-threshold subtraction is Sterbenz-exact so rounding can never
# flip the sign.
TSPAT = os.environ.get("LIF_TSPAT", "vvvvvv")
assert len(TSPAT) == T and set(TSPAT) <= set("vla"), TSPAT
TSPAT_B = {2: "vvavva", 3: "vaavva"}
if os.environ.get("LIF_TSPAT_B") is not None:
    TSPAT_B = {}
    for _it in os.environ.get("LIF_TSPAT_B", "").split(";"):
        if _it:
            _b, _pat = _it.split(":")
            assert len(_pat) == T and set(_pat) <= set("vla")
            TSPAT_B[int(_b)] = _pat


def ts_engine(blk, t):
    return TSPAT_B.get(blk, TSPAT)[t]


# Blocks whose 6-bit code is accumulated by DVE tensor_tensor adds in
# SBUF (skipping the PE pack + ScalarE PSUM copy in the tail chain).
DCODE = {int(v) for v in os.environ.get("LIF_DCODE", "").split(",") if v}

# Blocks whose output store is split per PSUM chunk so the first half
# streams out while the second half is still packing/copying.
# NOTE: measured as a small win (~300ns) in TimelineSim but produces
# wrong results on the real execution path (race not modeled by the
# cost model) -- keep OFF.
SPLIT_STORE = {int(v) for v in
               os.environ.get("LIF_SPLIT_STORE", "").split(",") if v}


_COMPILED = None
LAST_RESULTS = None


def _build_program():
    nc = bacc.Bacc(None, target_bir_lowering=False, debug=False)

    f32 = mybir.dt.float32
    f32r = mybir.dt.float32r
    f8 = mybir.dt.float8e4
    bf16 = mybir.dt.bfloat16
    u8 = mybir.dt.uint8
    A = mybir.AluOpType

    x_d = nc.dram_tensor("x", [T, N_PER_CORE], f32, kind="ExternalInput")
    c_d = nc.dram_tensor("c", [N_PER_CORE], u8, kind="ExternalOutput")
    x_r = x_d[:].rearrange("t (p f) -> t p f", p=P)
    c_r = c_d[:].rearrange("(p f) -> p f", p=P)

    need_pe_sub = any(sub_engine(b, t) in "pf"
                      for b in range(NBLK) for t in range(T - 1))

    with tile.TileContext(nc) as tc:
        with (
            tc.tile_pool(name="consts", bufs=1) as consts,
            tc.tile_pool(name="u", bufs=1) as u_pool,
            tc.tile_pool(name="w6", bufs=1) as w_pool,
            tc.tile_pool(name="cp", bufs=int(os.environ.get("LIF_CP_BUFS", "2")),
                         space="PSUM") as cp_pool,
            tc.tile_pool(name="cpl",
                         bufs=1, space="PSUM") as cpl_pool,
            tc.tile_pool(name="cs", bufs=int(os.environ.get("LIF_CS_BUFS", "2"))) as cs_pool,
            tc.tile_pool(name="up", bufs=int(os.environ.get("LIF_UP_BUFS", "3")),
                         space="PSUM") as up_pool,
            tc.tile_pool(name="st", bufs=1) as st_pool,
        ):
            ident = consts.tile([P, P], f8, name="ident")
            make_identity(nc, ident)
            if need_pe_sub:
                identb = consts.tile([P, P], bf16, name="identb")
                make_identity(nc, identb)
                identn = consts.tile([P, P], bf16, name="identn")
                nc.gpsimd.memset(identn[:], 0.0)
                nc.gpsimd.affine_select(
                    out=identn[:], in_=identn[:],
                    compare_op=mybir.AluOpType.not_equal, fill=-1.0,
                    base=0, pattern=[[-1, P]], channel_multiplier=1,
                )
            sbias = {}
            for _t in range(T):
                if any(ts_engine(b, _t) == "a" for b in range(NBLK)):
                    v = -(float(2.0 ** _t) - float(2.0 ** (_t - 24)))
                    sbias[_t] = consts.tile([P, 1], f32, name=f"sb{_t}")
                    nc.gpsimd.memset(sbias[_t][:], v)

            u = [None] * NBLK
            w6 = [None] * NBLK
            stg = [None] * NBLK
            stg_slot = [None] * NBLK
            code = [None] * NBLK
            cp_last = {}
            nlast = int(os.environ.get("LIF_NLAST", "2"))

            def author_compute(blk, t):
                """load (t=0) + spike compare for one block-step."""
                c0, fb = OFFS[blk], BLOCKS[blk]
                thr = float(2.0 ** t)
                if t == 0:
                    u[blk] = u_pool.tile([P, fb], f32, tag=f"u{blk}",
                                         name=f"u{blk}")
                    # U_0 = xs_0 (v0 = 0)
                    nc.sync.dma_start(out=u[blk][:],
                                      in_=x_r[0][:, c0:c0 + fb])
                    w6[blk] = w_pool.tile([P, T * fb], f8, tag=f"w6b{blk}",
                                          name=f"w6_{blk}")
                    sts = sorted(STAGED.get(blk, ()))
                    if sts:
                        stg_slot[blk] = {tl: i for i, tl in enumerate(sts)}
                        stg[blk] = st_pool.tile(
                            [P, len(sts) * fb], f32, tag=f"st{blk}",
                            name=f"st{blk}")
                # staged loads for planes due this step (lead-based);
                # optionally pinned late via a scheduler time hint so
                # they fill DMA gaps instead of displacing early accums
                for tl in sorted(STAGED.get(blk, ())):
                    if max(0, tl - STAGE_LEAD) == t:
                        c0_, fb_ = OFFS[blk], BLOCKS[blk]
                        i = stg_slot[blk][tl]
                        ms = (starts[blk] + tl - STAGE_LEAD) * \
                            STAGE_ROUND_US * 1e-3
                        with tc.tile_wait_until(ms, enable=ms > 0):
                            nc.sync.dma_start(
                                out=stg[blk][:, i * fb_:(i + 1) * fb_],
                                in_=x_r[tl][:, c0_:c0_ + fb_])
                sl = w6[blk][:, t * fb:(t + 1) * fb]
                # w = (U >= 2^t) * 2^t -> fp8e4 {0, 2^t}, both exact
                te = ts_engine(blk, t)
                if te == "a":
                    # sigma = sign(U - (2^t - 2^(t-24))) in {-1,0,1},
                    # then w = relu(sigma * 2^t) in {0, 2^t}
                    nc.scalar.activation(
                        out=sl, in_=u[blk][:],
                        func=mybir.ActivationFunctionType.Sign,
                        bias=sbias[t][:])
                    nc.scalar.activation(
                        out=sl, in_=sl,
                        func=mybir.ActivationFunctionType.Relu, scale=thr)
                else:
                    ts_ns = nc.vector if te == "v" else nc.gpsimd
                    ts_ns.tensor_scalar(
                        out=sl, in0=u[blk][:], scalar1=thr, scalar2=thr,
                        op0=A.is_ge, op1=A.mult,
                    )
                return sl

            def author_sub(blk, t, sl):
                """U -= w on the engine chosen for (blk, t)."""
                eng = sub_engine(blk, t)
                fb = BLOCKS[blk]
                if eng == "d":
                    nc.vector.tensor_tensor(
                        out=u[blk][:], in0=u[blk][:], in1=sl, op=A.subtract)
                elif eng == "l":
                    nc.gpsimd.tensor_tensor(
                        out=u[blk][:], in0=u[blk][:], in1=sl, op=A.subtract)
                else:  # 'p'/'f': PE identity matmuls, chunked at MM cols
                    # 'f' additionally folds U += xs_{t+1} (staged plane)
                    # into the same PSUM chain.  Rounding matches the
                    # reference exactly: psum accumulates one fp32
                    # rounding per matmul -> fp32(fp32(U - w) + xs).
                    urr = u[blk][:].bitcast(f32r)
                    fused = eng == "f"
                    if fused:
                        i = stg_slot[blk][t + 1]
                        srr = stg[blk][:, i * fb:(i + 1) * fb].bitcast(f32r)
                    for j0 in range(0, fb, MM):
                        m = min(MM, fb - j0)
                        up = up_pool.tile([P, MM], f32, tag="up",
                                          name=f"up{blk}_{t}_{j0}")
                        nc.tensor.matmul(
                            up[:, :m], identb[:], urr[:, j0:j0 + m],
                            start=True, stop=False)
                        nc.tensor.matmul(
                            up[:, :m], identn[:], sl[:, j0:j0 + m],
                            start=False, stop=not fused)
                        if fused:
                            nc.tensor.matmul(
                                up[:, :m], identb[:], srr[:, j0:j0 + m],
                                start=False, stop=True)
                        nc.scalar.copy(out=u[blk][:, j0:j0 + m],
                                       in_=up[:, :m])

            def author_accum(blk, t):
                c0, fb = OFFS[blk], BLOCKS[blk]
                if (t + 1) in STAGED.get(blk, ()):
                    # U += xs_{t+1} from the staged plane (engine add)
                    i = stg_slot[blk][t + 1]
                    add_ns = (nc.vector if STAGED_ENG.get(blk, "d") == "d"
                              else nc.gpsimd)
                    add_ns.tensor_tensor(
                        out=u[blk][:], in0=u[blk][:],
                        in1=stg[blk][:, i * fb:(i + 1) * fb], op=A.add)
                else:
                    # U += xs_{t+1}: accumulate during load (SWDGE CCE add)
                    nc.gpsimd.dma_start(
                        out=u[blk][:], in_=x_r[t + 1][:, c0:c0 + fb],
                        accum_op=A.add,
                    )

            def author_pack(blk, t):
                c0, fb = OFFS[blk], BLOCKS[blk]
                sl = w6[blk][:, t * fb:(t + 1) * fb]
                if blk in DCODE:
                    # code accumulated in SBUF by DVE adds (exact small
                    # ints); final step emits uint8 and stores directly
                    if t == 0:
                        code[blk] = st_pool.tile([P, fb], f32,
                                                 tag=f"cd{blk}",
                                                 name=f"cd{blk}")
                        nc.vector.tensor_scalar(
                            out=code[blk][:], in0=sl, scalar1=1.0,
                            scalar2=None, op0=A.mult)
                    elif t < T - 1:
                        nc.vector.tensor_tensor(
                            out=code[blk][:], in0=code[blk][:], in1=sl,
                            op=A.add)
                    else:
                        cs = cs_pool.tile([P, fb], u8, tag=f"cs{fb}",
                                          name=f"cs{blk}")
                        nc.vector.tensor_tensor(
                            out=cs[:], in0=code[blk][:], in1=sl,
                            op=A.add)
                        nc.sync.dma_start(out=c_r[:, c0:c0 + fb],
                                          in_=cs[:])
                    return
                last_blk = blk >= NBLK - nlast
                if last_blk:
                    # pack per step into a dedicated PSUM accumulator so
                    # only t=5's matmuls are in the tail
                    if t == 0:
                        cp_last[blk] = cpl_pool.tile(
                            [P, max(BLOCKS[NBLK - nlast:])], f32,
                            tag=f"cpl{blk}", name=f"cpl{blk}")
                    for j0 in range(0, fb, MM):
                        m = min(MM, fb - j0)
                        o = t * fb + j0
                        nc.tensor.matmul(
                            cp_last[blk][:, j0:j0 + m],
                            ident[:], w6[blk][:, o:o + m],
                            start=(t == 0), stop=(t == T - 1),
                        )
                    if t == T - 1:
                        cs = cs_pool.tile([P, fb], u8, tag=f"cs{fb}",
                                          name=f"cs{blk}")
                        if blk in SPLIT_STORE and fb >= 512:
                            h = fb // 2
                            for a, b in ((0, h), (h, fb)):
                                nc.scalar.copy(out=cs[:, a:b],
                                               in_=cp_last[blk][:, a:b])
                                nc.sync.dma_start(
                                    out=c_r[:, c0 + a:c0 + b],
                                    in_=cs[:, a:b])
                        else:
                            nc.scalar.copy(out=cs[:],
                                           in_=cp_last[blk][:, :fb])
                            nc.sync.dma_start(out=c_r[:, c0:c0 + fb],
                                              in_=cs[:])
                elif t == T - 1:
                    # end-of-block PE burst, chunked through small PSUM
                    # tiles so banks recycle quickly
                    cs = cs_pool.tile([P, fb], u8, tag=f"cs{fb}",
                                      name=f"cs{blk}")
                    split = blk in SPLIT_STORE
                    for j0 in range(0, fb, MM):
                        m = min(MM, fb - j0)
                        cp = cp_pool.tile([P, MM], f32, tag="cp",
                                          name=f"cp{blk}_{j0}")
                        for tt in range(T):
                            o = tt * fb + j0
                            nc.tensor.matmul(
                                cp[:, :m], ident[:], w6[blk][:, o:o + m],
                                start=(tt == 0), stop=(tt == T - 1),
                            )
                        nc.scalar.copy(out=cs[:, j0:j0 + m], in_=cp[:, :m])
                        if split:
                            nc.sync.dma_start(
                                out=c_r[:, c0 + j0:c0 + j0 + m],
                                in_=cs[:, j0:j0 + m])
                    if not split:
                        nc.sync.dma_start(out=c_r[:, c0:c0 + fb], in_=cs[:])

            # skewed rounds: block b runs step (r - SKEW*b); ops authored
            # inline per block (ts, sub, accum, pack)
            if os.environ.get("LIF_ORDER"):
                order = [int(v) for v in os.environ["LIF_ORDER"].split(",")]
                assert sorted(order) == list(range(NBLK))
            else:
                order = list(range(NBLK))
                if REV:
                    order = order[::-1]
            if os.environ.get("LIF_STARTS"):
                starts = [int(v) for v in
                          os.environ["LIF_STARTS"].split(",")]
                assert len(starts) == NBLK
            elif NBLK == 8 and not os.environ.get("LIF_SKEW"):
                starts = [0, 1, 2, 3, 4, 5, 6, 6]
            else:
                starts = [SKEW * b for b in range(NBLK)]
            for r in range(T + max(starts)):
                for blk in order:
                    t = r - starts[blk]
                    if not (0 <= t < T):
                        continue
                    sl = author_compute(blk, t)
                    if t < T - 1:
                        author_sub(blk, t, sl)
                        if sub_engine(blk, t) != "f":
                            author_accum(blk, t)
                    author_pack(blk, t)
    nc.finalize()
    return nc


_XS_SCALE = (2.0 ** np.arange(T, dtype=np.float32)).reshape(T, 1)


def kernel(input_current: np.ndarray) -> np.ndarray:
    global _COMPILED, LAST_RESULTS
    x = np.asarray(input_current, dtype=np.float32)
    assert x.shape == (T, B, C, H, W), x.shape

    if _COMPILED is None:
        _COMPILED = _build_program()
    nc = _COMPILED

    in_maps = []
    for k in range(N_CORES):
        shard = np.ascontiguousarray(
            x[:, k * B_PER_CORE:(k + 1) * B_PER_CORE]
        ).reshape(T, N_PER_CORE)
        # xs_t = 2^t * x_t (exact in fp32: pure exponent shift)
        in_maps.append({"x": shard * _XS_SCALE})

    trace = bool(int(os.environ.get("LIF_TRACE", "0")))
    res = run_bass_kernel_spmd(nc, in_maps, core_ids=list(range(N_CORES)),
                               trace=trace)
    LAST_RESULTS = res

    out = np.empty((T, B, C, H, W), dtype=np.float32)
    bits = np.arange(T, dtype=np.uint8).reshape(T, 1)
    for k in range(N_CORES):
        code = res.results[k]["c"]  # uint8 [N_PER_CORE]
        sp = ((code[None, :] >> bits) & 1).astype(np.float32)
        out[:, k * B_PER_CORE:(k + 1) * B_PER_CORE] = (
            sp.reshape(T, B_PER_CORE, C, H, W)
        )
    return out


# revision 45
# speedup vs baseline: 1.0106x; 1.0106x over previous
"""LIF spiking-neuron layer on 8 Trainium2 NeuronCores (Bass/Tile).

Reference semantics (per neuron, T=6 steps, v0=0):
    v = v*0.5 + x_t ; s = (v >= 1.0) ; v = v - s
Output: spikes [T, B, C, H, W] float32 (values are exactly 0.0 / 1.0).

Sharding: data-parallel over batch (axis 1): 64 batches / 8 cores.
Per core the neuron field (8*128*32*32 = 1,048,576 elements) is laid
out as [128 partitions, 8192 cols], processed in column blocks that are
software-pipelined with a one-timestep skew.

Scaled-state formulation (bit-identical to the fp32 reference):
  state U_t = 2^t * v_t; host pre-scales inputs xs_t = 2^t * x_t
  (exact power-of-2 scalings commute with fp32 round-to-nearest).
  Per step:
    w_t = (U_t >= 2^t) * 2^t    engine per (blk, t):
                                  'v' DVE tensor_scalar (2x perf mode)
                                  'a' ScalarE Sign+Relu pair: with bias
                                      -(2^t - 2^(t-24)) the largest fp32
                                      below 2^t maps to sign(0)=0 and the
                                      near-threshold subtract is Sterbenz-
                                      exact, so this is exact for ALL
                                      fp32 inputs
                                  'l' Pool tensor_scalar
    U  -=  w_t                  'd' DVE / 'l' Pool tensor_tensor /
                                'p'/'f' PE identity matmuls
    U  +=  xs_{t+1}             by the load DMA itself (SWDGE CCE add),
                                or a DVE/Pool add from a staged
                                dependency-free load (LIF_STAGE)
  The TensorEngine packs the 6 fp8 spike planes into a 6-bit code
  C = sum_t 2^t s_t via identity matmuls accumulated in PSUM; ScalarE
  copies PSUM->SBUF as uint8; the store writes 1 MiB/core.  The host
  unpacks bits to {0,1} f32.

Schedule (TimelineSim 86127 ns vs 88360 baseline; DMA busy 72.9 us):
  descending blocks [1536,...,384], starts [0..6,6] (last two blocks
  bunched), t0 subs of blocks 1,3,4,5,6 + t2 sub of block 2 on Pool,
  spike compares (2,t2),(2,t5),(3,t1),(3,t2),(3,t5) on ScalarE, x_1 of
  block 0 staged (breaks the fill-phase accum->compare latency loop),
  and per-step packing for the last 2 blocks (NLAST=2).  All remaining
  DMA idle is the ~1.9 us issue ramp at the start, ~2 us of fill, and
  the ~5 us drain cascade (last accum + 900ns sem + compare/pack/copy/
  store chain).
"""

import os
import sys

import numpy as np

sys.path.insert(0, "/opt/trn_rl_repo")

import concourse.bacc as bacc
import concourse.bass as bass
import concourse.mybir as mybir
from concourse import tile
from concourse.bass_utils import run_bass_kernel_spmd
from concourse.masks import make_identity

T = 6
B = 64
C = 128
H = 32
W = 32
N_CORES = 8
B_PER_CORE = B // N_CORES
N_PER_CORE = B_PER_CORE * C * H * W  # 1,048,576
P = 128
FTOT = N_PER_CORE // P               # 8192
if os.environ.get("LIF_BLOCKS"):
    BLOCKS = [int(v) for v in os.environ["LIF_BLOCKS"].split(",")]
    assert sum(BLOCKS) == FTOT, BLOCKS
elif os.environ.get("LIF_NBLK"):
    _n = int(os.environ["LIF_NBLK"])
    BLOCKS = [FTOT // _n] * _n
else:
    BLOCKS = [1536, 1280, 1024, 1152, 1024, 1024, 512, 640]
NBLK = len(BLOCKS)
OFFS = [sum(BLOCKS[:i]) for i in range(NBLK)]
MM = 512                             # PE moving-free / PSUM chunk
SKEW = int(os.environ.get("LIF_SKEW", "1"))
REV = int(os.environ.get("LIF_REV", "1"))

# Subtract-engine pattern: one char per t in 0..4 ('d'=DVE, 'p'=PE,
# 'l'=Pool, 'f'=PE-fused sub+add consuming a staged x_{t+1} plane),
# applied to every block; per-block override via
# LIF_SUBPAT_B="blk:pattern;..."
SUBPAT = os.environ.get("LIF_SUBPAT", "ddddd")
assert len(SUBPAT) == T - 1 and set(SUBPAT) <= set("dplf"), SUBPAT
SUBPAT_B = {1: "ldddd", 2: "ddldd", 3: "ldddd", 4: "ldddd", 5: "ldddd",
            6: "ldddd"}
if os.environ.get("LIF_SUBPAT_B") is not None:
    SUBPAT_B = {}
    for it in os.environ.get("LIF_SUBPAT_B", "").split(";"):
        if it:
            b_, pat_ = it.split(":")
            assert len(pat_) == T - 1 and set(pat_) <= set("dplf")
            SUBPAT_B[int(b_)] = pat_

# How many steps ahead of use a staged plane's load is authored.
STAGE_LEAD = int(os.environ.get("LIF_STAGE_LEAD", "2"))
# Scheduler time hint: staged load for (blk, tl) is pinned to fire no
# earlier than (starts[blk] + tl - STAGE_LEAD) * STAGE_ROUND_US.
STAGE_ROUND_US = float(os.environ.get("LIF_STAGE_ROUND_US", "0"))


def sub_engine(blk, t):
    return SUBPAT_B.get(blk, SUBPAT)[t]


# Staged timesteps: "blk:ts[:eng];..." e.g. "6:45:l;7:12345" stages
# x_4,x_5 of block 6 (adds on Pool) and x_1..x_5 of block 7 (adds on
# DVE, the default) as dependency-free loads; the U += xs for those
# steps runs as a tensor_tensor add instead of an accum DMA,
# collapsing the drain-chain latency.
STAGED = {0: {1}}
STAGED_ENG = {0: "d"}
if os.environ.get("LIF_STAGE") is not None:
    STAGED = {}
    STAGED_ENG = {}
    for _it in os.environ.get("LIF_STAGE", "").split(";"):
        if _it:
            _parts = _it.split(":")
            _b = int(_parts[0])
            STAGED[_b] = {int(ch) for ch in _parts[1]}
            STAGED_ENG[_b] = _parts[2] if len(_parts) > 2 else "d"

# 'f' sub steps consume staged x_{t+1}: add them to the staged set
for _b in range(NBLK):
    for _t in range(T - 1):
        if SUBPAT_B.get(_b, SUBPAT)[_t] == "f":
            STAGED.setdefault(_b, set()).add(_t + 1)
            STAGED_ENG.setdefault(_b, "d")

# Spike-compare engine per (blk, t in 0..5): 'v' DVE (2x perf mode),
# 'l' Pool, 'a' ScalarE/Activation via Sign+Relu.  The 'a' path is
# exact: with bias -(2^t - 2^(t-24)), the largest fp32 below 2^t maps
# to sign(0)=0 (no spike) and every U >= 2^t maps to sign(+)=1; the
# near-threshold subtraction is Sterbenz-exact so rounding can never
# flip the sign.
TSPAT = os.environ.get("LIF_TSPAT", "vvvvvv")
assert len(TSPAT) == T and set(TSPAT) <= set("vla"), TSPAT
TSPAT_B = {2: "vvavva", 3: "vaavva"}
if os.environ.get("LIF_TSPAT_B") is not None:
    TSPAT_B = {}
    for _it in os.environ.get("LIF_TSPAT_B", "").split(";"):
        if _it:
            _b, _pat = _it.split(":")
            assert len(_pat) == T and set(_pat) <= set("vla")
            TSPAT_B[int(_b)] = _pat


def ts_engine(blk, t):
    return TSPAT_B.get(blk, TSPAT)[t]


# Blocks whose 6-bit code is accumulated by DVE tensor_tensor adds in
# SBUF (skipping the PE pack + ScalarE PSUM copy in the tail chain).
DCODE = {int(v) for v in os.environ.get("LIF_DCODE", "").split(",") if v}

# Blocks whose output store is split per PSUM chunk so the first half
# streams out while the second half is still packing/copying.
# NOTE: measured as a small win (~300ns) in TimelineSim but produces
# wrong results on the real execution path (race not modeled by the
# cost model) -- keep OFF.
SPLIT_STORE = {int(v) for v in
               os.environ.get("LIF_SPLIT_STORE", "").split(",") if v}

# Merge the last two blocks' output stores into a single contiguous
# store (they finish in the same round with bunched starts); removes
# one SP store-issue chain from the drain cascade.
MERGE_TAIL = bool(int(os.environ.get("LIF_MERGE_TAIL", "1")))


_COMPILED = None
LAST_RESULTS = None


def _build_program():
    nc = bacc.Bacc(None, target_bir_lowering=False, debug=False)

    f32 = mybir.dt.float32
    f32r = mybir.dt.float32r
    f8 = mybir.dt.float8e4
    bf16 = mybir.dt.bfloat16
    u8 = mybir.dt.uint8
    A = mybir.AluOpType

    x_d = nc.dram_tensor("x", [T, N_PER_CORE], f32, kind="ExternalInput")
    c_d = nc.dram_tensor("c", [N_PER_CORE], u8, kind="ExternalOutput")
    x_r = x_d[:].rearrange("t (p f) -> t p f", p=P)
    c_r = c_d[:].rearrange("(p f) -> p f", p=P)

    need_pe_sub = any(sub_engine(b, t) in "pf"
                      for b in range(NBLK) for t in range(T - 1))

    with tile.TileContext(nc) as tc:
        with (
            tc.tile_pool(name="consts", bufs=1) as consts,
            tc.tile_pool(name="u", bufs=1) as u_pool,
            tc.tile_pool(name="w6", bufs=1) as w_pool,
            tc.tile_pool(name="cp", bufs=int(os.environ.get("LIF_CP_BUFS", "2")),
                         space="PSUM") as cp_pool,
            tc.tile_pool(name="cpl",
                         bufs=1, space="PSUM") as cpl_pool,
            tc.tile_pool(name="cs", bufs=int(os.environ.get("LIF_CS_BUFS", "2"))) as cs_pool,
            tc.tile_pool(name="up", bufs=int(os.environ.get("LIF_UP_BUFS", "3")),
                         space="PSUM") as up_pool,
            tc.tile_pool(name="st", bufs=1) as st_pool,
        ):
            ident = consts.tile([P, P], f8, name="ident")
            make_identity(nc, ident)
            if need_pe_sub:
                identb = consts.tile([P, P], bf16, name="identb")
                make_identity(nc, identb)
                identn = consts.tile([P, P], bf16, name="identn")
                nc.gpsimd.memset(identn[:], 0.0)
                nc.gpsimd.affine_select(
                    out=identn[:], in_=identn[:],
                    compare_op=mybir.AluOpType.not_equal, fill=-1.0,
                    base=0, pattern=[[-1, P]], channel_multiplier=1,
                )
            sbias = {}
            for _t in range(T):
                if any(ts_engine(b, _t) == "a" for b in range(NBLK)):
                    v = -(float(2.0 ** _t) - float(2.0 ** (_t - 24)))
                    sbias[_t] = consts.tile([P, 1], f32, name=f"sb{_t}")
                    nc.gpsimd.memset(sbias[_t][:], v)

            u = [None] * NBLK
            w6 = [None] * NBLK
            stg = [None] * NBLK
            stg_slot = [None] * NBLK
            code = [None] * NBLK
            cs_merge = {}
            cp_last = {}
            nlast = int(os.environ.get("LIF_NLAST", "2"))

            def author_compute(blk, t):
                """load (t=0) + spike compare for one block-step."""
                c0, fb = OFFS[blk], BLOCKS[blk]
                thr = float(2.0 ** t)
                if t == 0:
                    u[blk] = u_pool.tile([P, fb], f32, tag=f"u{blk}",
                                         name=f"u{blk}")
                    # U_0 = xs_0 (v0 = 0)
                    nc.sync.dma_start(out=u[blk][:],
                                      in_=x_r[0][:, c0:c0 + fb])
                    w6[blk] = w_pool.tile([P, T * fb], f8, tag=f"w6b{blk}",
                                          name=f"w6_{blk}")
                    sts = sorted(STAGED.get(blk, ()))
                    if sts:
                        stg_slot[blk] = {tl: i for i, tl in enumerate(sts)}
                        stg[blk] = st_pool.tile(
                            [P, len(sts) * fb], f32, tag=f"st{blk}",
                            name=f"st{blk}")
                # staged loads for planes due this step (lead-based);
                # optionally pinned late via a scheduler time hint so
                # they fill DMA gaps instead of displacing early accums
                for tl in sorted(STAGED.get(blk, ())):
                    if max(0, tl - STAGE_LEAD) == t:
                        c0_, fb_ = OFFS[blk], BLOCKS[blk]
                        i = stg_slot[blk][tl]
                        ms = (starts[blk] + tl - STAGE_LEAD) * \
                            STAGE_ROUND_US * 1e-3
                        with tc.tile_wait_until(ms, enable=ms > 0):
                            nc.sync.dma_start(
                                out=stg[blk][:, i * fb_:(i + 1) * fb_],
                                in_=x_r[tl][:, c0_:c0_ + fb_])
                sl = w6[blk][:, t * fb:(t + 1) * fb]
                # w = (U >= 2^t) * 2^t -> fp8e4 {0, 2^t}, both exact
                te = ts_engine(blk, t)
                if te == "a":
                    # sigma = sign(U - (2^t - 2^(t-24))) in {-1,0,1},
                    # then w = relu(sigma * 2^t) in {0, 2^t}
                    nc.scalar.activation(
                        out=sl, in_=u[blk][:],
                        func=mybir.ActivationFunctionType.Sign,
                        bias=sbias[t][:])
                    nc.scalar.activation(
                        out=sl, in_=sl,
                        func=mybir.ActivationFunctionType.Relu, scale=thr)
                else:
                    ts_ns = nc.vector if te == "v" else nc.gpsimd
                    ts_ns.tensor_scalar(
                        out=sl, in0=u[blk][:], scalar1=thr, scalar2=thr,
                        op0=A.is_ge, op1=A.mult,
                    )
                return sl

            def author_sub(blk, t, sl):
                """U -= w on the engine chosen for (blk, t)."""
                eng = sub_engine(blk, t)
                fb = BLOCKS[blk]
                if eng == "d":
                    nc.vector.tensor_tensor(
                        out=u[blk][:], in0=u[blk][:], in1=sl, op=A.subtract)
                elif eng == "l":
                    nc.gpsimd.tensor_tensor(
                        out=u[blk][:], in0=u[blk][:], in1=sl, op=A.subtract)
                else:  # 'p'/'f': PE identity matmuls, chunked at MM cols
                    # 'f' additionally folds U += xs_{t+1} (staged plane)
                    # into the same PSUM chain.  Rounding matches the
                    # reference exactly: psum accumulates one fp32
                    # rounding per matmul -> fp32(fp32(U - w) + xs).
                    urr = u[blk][:].bitcast(f32r)
                    fused = eng == "f"
                    if fused:
                        i = stg_slot[blk][t + 1]
                        srr = stg[blk][:, i * fb:(i + 1) * fb].bitcast(f32r)
                    for j0 in range(0, fb, MM):
                        m = min(MM, fb - j0)
                        up = up_pool.tile([P, MM], f32, tag="up",
                                          name=f"up{blk}_{t}_{j0}")
                        nc.tensor.matmul(
                            up[:, :m], identb[:], urr[:, j0:j0 + m],
                            start=True, stop=False)
                        nc.tensor.matmul(
                            up[:, :m], identn[:], sl[:, j0:j0 + m],
                            start=False, stop=not fused)
                        if fused:
                            nc.tensor.matmul(
                                up[:, :m], identb[:], srr[:, j0:j0 + m],
                                start=False, stop=True)
                        nc.scalar.copy(out=u[blk][:, j0:j0 + m],
                                       in_=up[:, :m])

            def author_accum(blk, t):
                c0, fb = OFFS[blk], BLOCKS[blk]
                if (t + 1) in STAGED.get(blk, ()):
                    # U += xs_{t+1} from the staged plane (engine add)
                    i = stg_slot[blk][t + 1]
                    add_ns = (nc.vector if STAGED_ENG.get(blk, "d") == "d"
                              else nc.gpsimd)
                    add_ns.tensor_tensor(
                        out=u[blk][:], in0=u[blk][:],
                        in1=stg[blk][:, i * fb:(i + 1) * fb], op=A.add)
                else:
                    # U += xs_{t+1}: accumulate during load (SWDGE CCE add)
                    nc.gpsimd.dma_start(
                        out=u[blk][:], in_=x_r[t + 1][:, c0:c0 + fb],
                        accum_op=A.add,
                    )

            def author_pack(blk, t):
                c0, fb = OFFS[blk], BLOCKS[blk]
                sl = w6[blk][:, t * fb:(t + 1) * fb]
                if blk in DCODE:
                    # code accumulated in SBUF by DVE adds (exact small
                    # ints); final step emits uint8 and stores directly
                    if t == 0:
                        code[blk] = st_pool.tile([P, fb], f32,
                                                 tag=f"cd{blk}",
                                                 name=f"cd{blk}")
                        nc.vector.tensor_scalar(
                            out=code[blk][:], in0=sl, scalar1=1.0,
                            scalar2=None, op0=A.mult)
                    elif t < T - 1:
                        nc.vector.tensor_tensor(
                            out=code[blk][:], in0=code[blk][:], in1=sl,
                            op=A.add)
                    else:
                        cs = cs_pool.tile([P, fb], u8, tag=f"cs{fb}",
                                          name=f"cs{blk}")
                        nc.vector.tensor_tensor(
                            out=cs[:], in0=code[blk][:], in1=sl,
                            op=A.add)
                        nc.sync.dma_start(out=c_r[:, c0:c0 + fb],
                                          in_=cs[:])
                    return
                last_blk = blk >= NBLK - nlast
                if last_blk:
                    # pack per step into a dedicated PSUM accumulator so
                    # only t=5's matmuls are in the tail
                    if t == 0:
                        cp_last[blk] = cpl_pool.tile(
                            [P, max(BLOCKS[NBLK - nlast:])], f32,
                            tag=f"cpl{blk}", name=f"cpl{blk}")
                    for j0 in range(0, fb, MM):
                        m = min(MM, fb - j0)
                        o = t * fb + j0
                        nc.tensor.matmul(
                            cp_last[blk][:, j0:j0 + m],
                            ident[:], w6[blk][:, o:o + m],
                            start=(t == 0), stop=(t == T - 1),
                        )
                    if t == T - 1 and MERGE_TAIL and blk >= NBLK - 2:
                        fb_lo = BLOCKS[NBLK - 2]
                        fb_hi = BLOCKS[NBLK - 1]
                        if "tail" not in cs_merge:
                            cs_merge["tail"] = cs_pool.tile(
                                [P, fb_lo + fb_hi], u8, tag="cstail",
                                name="cstail")
                            cs_merge["done"] = 0
                        cst = cs_merge["tail"]
                        off = 0 if blk == NBLK - 2 else fb_lo
                        nc.scalar.copy(out=cst[:, off:off + fb],
                                       in_=cp_last[blk][:, :fb])
                        cs_merge["done"] += 1
                        if cs_merge["done"] == 2:
                            c_lo = OFFS[NBLK - 2]
                            nc.sync.dma_start(
                                out=c_r[:, c_lo:c_lo + fb_lo + fb_hi],
                                in_=cst[:])
                        return
                    if t == T - 1:
                        cs = cs_pool.tile([P, fb], u8, tag=f"cs{fb}",
                                          name=f"cs{blk}")
                        if blk in SPLIT_STORE and fb >= 512:
                            h = fb // 2
                            for a, b in ((0, h), (h, fb)):
                                nc.scalar.copy(out=cs[:, a:b],
                                               in_=cp_last[blk][:, a:b])
                                nc.sync.dma_start(
                                    out=c_r[:, c0 + a:c0 + b],
                                    in_=cs[:, a:b])
                        else:
                            nc.scalar.copy(out=cs[:],
                                           in_=cp_last[blk][:, :fb])
                            nc.sync.dma_start(out=c_r[:, c0:c0 + fb],
                                              in_=cs[:])
                elif t == T - 1:
                    # end-of-block PE burst, chunked through small PSUM
                    # tiles so banks recycle quickly
                    cs = cs_pool.tile([P, fb], u8, tag=f"cs{fb}",
                                      name=f"cs{blk}")
                    split = blk in SPLIT_STORE
                    for j0 in range(0, fb, MM):
                        m = min(MM, fb - j0)
                        cp = cp_pool.tile([P, MM], f32, tag="cp",
                                          name=f"cp{blk}_{j0}")
                        for tt in range(T):
                            o = tt * fb + j0
                            nc.tensor.matmul(
                                cp[:, :m], ident[:], w6[blk][:, o:o + m],
                                start=(tt == 0), stop=(tt == T - 1),
                            )
                        nc.scalar.copy(out=cs[:, j0:j0 + m], in_=cp[:, :m])
                        if split:
                            nc.sync.dma_start(
                                out=c_r[:, c0 + j0:c0 + j0 + m],
                                in_=cs[:, j0:j0 + m])
                    if not split:
                        nc.sync.dma_start(out=c_r[:, c0:c0 + fb], in_=cs[:])

            # skewed rounds: block b runs step (r - SKEW*b); ops authored
            # inline per block (ts, sub, accum, pack)
            if os.environ.get("LIF_ORDER"):
                order = [int(v) for v in os.environ["LIF_ORDER"].split(",")]
                assert sorted(order) == list(range(NBLK))
            else:
                order = list(range(NBLK))
                if REV:
                    order = order[::-1]
            if os.environ.get("LIF_STARTS"):
                starts = [int(v) for v in
                          os.environ["LIF_STARTS"].split(",")]
                assert len(starts) == NBLK
            elif NBLK == 8 and not os.environ.get("LIF_SKEW"):
                starts = [0, 1, 2, 3, 4, 5, 6, 6]
            else:
                starts = [SKEW * b for b in range(NBLK)]
            for r in range(T + max(starts)):
                for blk in order:
                    t = r - starts[blk]
                    if not (0 <= t < T):
                        continue
                    sl = author_compute(blk, t)
                    if t < T - 1:
                        author_sub(blk, t, sl)
                        if sub_engine(blk, t) != "f":
                            author_accum(blk, t)
                    author_pack(blk, t)
    nc.finalize()
    return nc


_XS_SCALE = (2.0 ** np.arange(T, dtype=np.float32)).reshape(T, 1)


def kernel(input_current: np.ndarray) -> np.ndarray:
    global _COMPILED, LAST_RESULTS
    x = np.asarray(input_current, dtype=np.float32)
    assert x.shape == (T, B, C, H, W), x.shape

    if _COMPILED is None:
        _COMPILED = _build_program()
    nc = _COMPILED

    in_maps = []
    for k in range(N_CORES):
        shard = np.ascontiguousarray(
            x[:, k * B_PER_CORE:(k + 1) * B_PER_CORE]
        ).reshape(T, N_PER_CORE)
        # xs_t = 2^t * x_t (exact in fp32: pure exponent shift)
        in_maps.append({"x": shard * _XS_SCALE})

    trace = bool(int(os.environ.get("LIF_TRACE", "0")))
    res = run_bass_kernel_spmd(nc, in_maps, core_ids=list(range(N_CORES)),
                               trace=trace)
    LAST_RESULTS = res

    out = np.empty((T, B, C, H, W), dtype=np.float32)
    bits = np.arange(T, dtype=np.uint8).reshape(T, 1)
    for k in range(N_CORES):
        code = res.results[k]["c"]  # uint8 [N_PER_CORE]
        sp = ((code[None, :] >> bits) & 1).astype(np.float32)
        out[:, k * B_PER_CORE:(k + 1) * B_PER_CORE] = (
            sp.reshape(T, B_PER_CORE, C, H, W)
        )
    return out
